# revision 1
# baseline (speedup 1.0000x reference)
"""nn_LSTETransformer kernel for 8 trn2 NeuronCores.

Sharding: vocab-parallel LM head on device (each core dequantizes its
4000-row shard of the ternary LM weight, transposes it on the PE, and runs
the [2048,1024]x[1024,4000] logits GEMM in bf16). The 4 transformer layers
run host-side in fp32 (mirror of the reference math).

Self-contained: only imports concourse (on sys.path in this container).
"""

import numpy as np

import concourse.bass as bass
import concourse.mybir as mybir
import concourse.tile as tile
from concourse.bass import ts
from concourse.bass_utils import run_bass_kernel_spmd
from concourse.masks import make_identity

N_CORES = 8
B, S, D, H, DFF, V, L = 2, 1024, 1024, 16, 4096, 32000, 4
GS = 128
DH = D // H
TOK = B * S            # 2048
VSH = V // N_CORES     # 4000
FT = D // 128          # 8 feature tiles

LAST_EXEC_NS = None

# ---------------------------------------------------------------- device part


def _build_lm_kernel():
    """Per-core: logits[2048, VSH] = bf16( h[2048,1024] ) @ deq(lm shard).T"""
    nc = bass.Bass()
    h_in = nc.declare_dram_parameter("h", [TOK, D], mybir.dt.float32, isOutput=False)
    lmt = nc.declare_dram_parameter("lm_t", [VSH, D], mybir.dt.int8, isOutput=False)
    lms = nc.declare_dram_parameter("lm_s", [VSH, D // GS], mybir.dt.float32, isOutput=False)
    out = nc.declare_dram_parameter("logits", [TOK, VSH], mybir.dt.float32, isOutput=True)

    bf16 = mybir.dt.bfloat16
    f32 = mybir.dt.float32

    with tile.TileContext(nc) as tc:
        with (
            tc.tile_pool(name="const", bufs=1) as constp,
            tc.tile_pool(name="persist", bufs=1) as persist,
            tc.tile_pool(name="htmp", bufs=3) as htmp,
            tc.tile_pool(name="wprep", bufs=3) as wprep,
            tc.tile_pool(name="lmch", bufs=2) as lmch,
            tc.tile_pool(name="ost", bufs=4) as ostp,
            tc.tile_pool(name="pst", bufs=2, space="PSUM") as pst,
            tc.tile_pool(name="psl", bufs=3, space="PSUM") as psl,
        ):
            ident = constp.tile([128, 128], bf16)
            make_identity(nc, ident[:])

            # hT_sb[p, ft, t] = h[t, ft*128+p]  (bf16)
            hT = persist.tile([128, FT, TOK], bf16)
            for tt in range(TOK // 128):
                hn = htmp.tile([128, D], f32, tag="hn")
                nc.sync.dma_start(out=hn[:], in_=h_in[ts(tt, 128), :])
                hb = htmp.tile([128, D], bf16, tag="hb")
                nc.scalar.copy(out=hb[:], in_=hn[:])
                pt = pst.tile([128, FT, 128], bf16, tag="pt")
                for ft in range(FT):
                    nc.tensor.transpose(
                        out=pt[:, ft, :], in_=hb[:, ts(ft, 128)], identity=ident[:]
                    )
                nc.scalar.copy(out=hT[:, :, ts(tt, 128)], in_=pt[:])

            # LM head: vocab chunks of 512
            n_vc = (VSH + 511) // 512
            for vc in range(n_vc):
                vw = min(512, VSH - vc * 512)
                lmT = lmch.tile([128, FT, 512], bf16, tag="lmT")
                for o4 in range((vw + 127) // 128):
                    r0 = vc * 512 + o4 * 128
                    nr = min(128, VSH - r0)
                    codes = wprep.tile([128, D], mybir.dt.int8, tag="codes")
                    nc.sync.dma_start(out=codes[:nr, :], in_=lmt[r0 : r0 + nr, :])
                    scl = wprep.tile([128, D // GS], f32, tag="scl")
                    nc.sync.dma_start(out=scl[:nr, :], in_=lms[r0 : r0 + nr, :])
                    wdq = wprep.tile([128, D], bf16, tag="wdq")
                    for g in range(D // GS):
                        nc.vector.tensor_scalar_mul(
                            wdq[:nr, ts(g, 128)],
                            codes[:nr, ts(g, 128)],
                            scl[:nr, g : g + 1],
                        )
                    ptw = pst.tile([128, FT, 128], bf16, tag="ptw")
                    for kt in range(FT):
                        nc.tensor.transpose(
                            out=ptw[:, kt, :nr],
                            in_=wdq[:nr, ts(kt, 128)],
                            identity=ident[:nr, :nr],
                        )
                    nc.scalar.copy(
                        out=lmT[:, :, o4 * 128 : o4 * 128 + nr], in_=ptw[:, :, :nr]
                    )
                for tt in range(TOK // 128):
                    pl = psl.tile([128, 512], f32, tag="pl")
                    for kt in range(FT):
                        nc.tensor.matmul(
                            out=pl[:, :vw],
                            lhsT=hT[:, kt, ts(tt, 128)],
                            rhs=lmT[:, kt, :vw],
                            start=(kt == 0),
                            stop=(kt == FT - 1),
                        )
                    ot = ostp.tile([128, 512], f32, tag="ot")
                    nc.scalar.copy(out=ot[:, :vw], in_=pl[:, :vw])
                    nc.sync.dma_start(
                        out=out[ts(tt, 128), vc * 512 : vc * 512 + vw],
                        in_=ot[:, :vw],
                    )
    _split_excess_waits(nc)
    return nc


def _split_excess_waits(nc, max_waits=1):
    """walrus here rejects >1 sem-wait per instruction; hoist extras onto NOPs."""
    for fn in nc.m.functions:
        for blk in fn.blocks:
            new_insts, dirty = [], False
            for inst in blk.instructions:
                si = inst.sync_info
                if si is not None and si.on_wait and len(si.on_wait) > max_waits:
                    waits = list(si.on_wait)
                    excess, keep = waits[:-max_waits], waits[-max_waits:]
                    for i in range(0, len(excess), max_waits):
                        new_insts.append(
                            mybir.InstNoOp(
                                name=f"{inst.name}-waitsplit-{i}",
                                engine=inst.engine,
                                sync_info=mybir.SyncInfo(
                                    on_wait=excess[i : i + max_waits], on_update=[]
                                ),
                                text_hint="waitsplit",
                                bass_nofuse=True,
                            )
                        )
                    inst.sync_info = mybir.SyncInfo(
                        on_wait=keep, on_update=list(si.on_update)
                    )
                    dirty = True
                new_insts.append(inst)
            if dirty:
                blk.instructions = new_insts


_NC_CACHE = None


def _get_nc():
    global _NC_CACHE
    if _NC_CACHE is None:
        _NC_CACHE = _build_lm_kernel()
    return _NC_CACHE


# ----------------------------------------------------------------- host part


def _deq(t, s):
    t = np.asarray(t, np.float32)
    return (t.reshape(-1, GS) * np.asarray(s, np.float32).reshape(-1, 1)).reshape(
        t.shape
    )


def _rmsnorm(x, w, eps=1e-6):
    ms = np.mean(x * x, axis=-1, keepdims=True, dtype=np.float32)
    return x * (1.0 / np.sqrt(ms + eps)) * w


def _softmax(a):
    a = a - a.max(axis=-1, keepdims=True)
    e = np.exp(a)
    return e / e.sum(axis=-1, keepdims=True)


def _host_layers(inp):
    ids = np.asarray(inp["input_ids"])
    x = _deq(inp["emb_t"], inp["emb_s"])[ids]  # [B,S,D]
    scale = DH**-0.5
    causal = np.tril(np.ones((S, S), dtype=bool))
    alpha = np.asarray(inp["alpha"], np.float32)
    for i in range(L):
        h = _rmsnorm(x, np.asarray(inp["na_w"])[i])
        wq = _deq(inp["wq_t"][i], inp["wq_s"][i])
        wk = _deq(inp["wk_t"][i], inp["wk_s"][i])
        wv = _deq(inp["wv_t"][i], inp["wv_s"][i])
        q = (h @ wq.T).reshape(B, S, H, DH).transpose(0, 2, 1, 3)
        k = (h @ wk.T).reshape(B, S, H, DH).transpose(0, 2, 1, 3)
        v = (h @ wv.T).reshape(B, S, H, DH).transpose(0, 2, 1, 3)
        att = np.einsum("bhqd,bhkd->bhqk", q, k) * scale
        att = np.where(causal, att, np.finfo(np.float32).min)
        p = _softmax(att)
        o = np.einsum("bhqk,bhkd->bhqd", p, v)
        xh = h.reshape(B, S, H, DH).transpose(0, 2, 1, 3)
        o = o + alpha[i][None, :, None, None] * xh
        o = o.transpose(0, 2, 1, 3).reshape(B, S, D)
        x = x + o @ _deq(inp["wo_t"][i], inp["wo_s"][i]).T
        h = _rmsnorm(x, np.asarray(inp["nm_w"])[i])
        g = h @ _deq(inp["wg_t"][i], inp["wg_s"][i]).T
        u = h @ _deq(inp["wu_t"][i], inp["wu_s"][i]).T
        silu = g / (1.0 + np.exp(-g))
        x = x + (silu * u) @ _deq(inp["wd_t"][i], inp["wd_s"][i]).T
    x = _rmsnorm(x, np.asarray(inp["fn_w"]))
    return x.reshape(TOK, D).astype(np.float32)


# ----------------------------------------------------------------- entry


def kernel(_trace=False, **inputs):
    global LAST_EXEC_NS
    inputs = {k: np.asarray(v) for k, v in inputs.items()}
    h_fin = _host_layers(inputs)

    lm_t = np.asarray(inputs["lm_t"], np.int8)
    lm_s = np.asarray(inputs["lm_s"], np.float32).reshape(V, D // GS)

    in_maps = []
    for c in range(N_CORES):
        r0 = c * VSH
        in_maps.append(
            {
                "h": h_fin,
                "lm_t": lm_t[r0 : r0 + VSH],
                "lm_s": lm_s[r0 : r0 + VSH],
            }
        )

    nc = _get_nc()
    res = run_bass_kernel_spmd(
        nc, in_maps, list(range(N_CORES)), trace=bool(_trace)
    )
    if getattr(res, "exec_time_ns", None):
        LAST_EXEC_NS = res.exec_time_ns
    logits = np.concatenate(
        [res.results[c]["logits"] for c in range(N_CORES)], axis=1
    )
    return logits.reshape(B, S, V).astype(np.float32)



# revision 22
# speedup vs baseline: 3.4608x; 3.4608x over previous
"""nn_LSTETransformer kernel for 8 trn2 NeuronCores.

Full transformer on-device. Sharding (tensor-parallel per hint):
- Attention: 2 heads per core (Wq/Wk/Wv row-shards of 128), Wo column-shard
  [1024,128], AllReduce(f32) of the output-projection partials per layer.
- MLP: wg/wu row-shards of 512, wd column-shard [1024,512], AllReduce per layer.
- LM head: vocab-shard 4000 rows/core, logits returned transposed in fp16.

Activations live in transposed layout xT/hT [D-part, token-free]; all weight
and probability transposes use the DMA xbar transpose (out[p,g,q] = in[q,g*128+p]).

Self-contained: only imports concourse (on sys.path in this container).
"""

import numpy as np

try:  # persistent XLA compilation cache saves a few seconds per fresh process
    import jax as _jax
    _jax.config.update("jax_compilation_cache_dir", "/root/.jax_comp_cache")
    _jax.config.update("jax_persistent_cache_min_entry_size_bytes", -1)
    _jax.config.update("jax_persistent_cache_min_compile_time_secs", 0)
except Exception:
    pass

import concourse.bass as bass
import concourse.mybir as mybir
import concourse.tile as tile
from concourse.bass_utils import run_bass_kernel_spmd

N_CORES = 8
B, S, D, H, DFF, V, L = 2, 1024, 1024, 16, 4096, 32000, 4
GS = 128
DH = D // H            # 64
TOK = B * S            # 2048
VSH = V // N_CORES     # 4000
KT = D // 128          # 8 feature tiles
FSH = DFF // N_CORES   # 512 ff rows per core
HL = H // N_CORES      # 2 heads per core

f32 = mybir.dt.float32
f16 = mybir.dt.float16
i8 = mybir.dt.int8
u8 = mybir.dt.uint8
AF = mybir.ActivationFunctionType
ALU = mybir.AluOpType
AX = mybir.AxisListType

LAST_EXEC_NS = None

# ---------------------------------------------------------------- device part


def _split_excess_waits(nc, max_waits=1):
    """walrus here rejects >1 sem-wait per instruction; hoist extras onto NOPs."""
    for fn in nc.m.functions:
        for blk in fn.blocks:
            new_insts, dirty = [], False
            for inst in blk.instructions:
                si = inst.sync_info
                if si is not None and si.on_wait and len(si.on_wait) > max_waits:
                    waits = list(si.on_wait)
                    excess, keep = waits[:-max_waits], waits[-max_waits:]
                    for i in range(0, len(excess), max_waits):
                        new_insts.append(
                            mybir.InstNoOp(
                                name=f"{inst.name}-waitsplit-{i}",
                                engine=inst.engine,
                                sync_info=mybir.SyncInfo(
                                    on_wait=excess[i : i + max_waits], on_update=[]
                                ),
                                text_hint="waitsplit",
                                bass_nofuse=True,
                            )
                        )
                    inst.sync_info = mybir.SyncInfo(
                        on_wait=keep, on_update=list(si.on_update)
                    )
                    dirty = True
                new_insts.append(inst)
            if dirty:
                blk.instructions = new_insts


def _rmsnorm(nc, tc, epsb, xT, hT, ncol, ones_col, ones_row):
    """hT[:,k,t] = xT[:,k,t] * rsqrt(mean_D(x^2)+eps) * ncol[:,k] (f16 out)."""
    with (
        tc.tile_pool(name="nrm_sb", bufs=2) as nsb,
        tc.tile_pool(name="nrm_ps", bufs=1, space="PSUM") as nps,
    ):
        ssq = nps.tile([1, 4, 512], f32)
        for k in range(KT):
            sq = nsb.tile([128, 2048], f16, tag="sq")
            nc.vector.tensor_mul(out=sq[:], in0=xT[:, k, :], in1=xT[:, k, :])
            for ch in range(4):
                nc.tensor.matmul(
                    out=ssq[:, ch, :],
                    lhsT=ones_col[:],
                    rhs=sq[:, ch * 512 : (ch + 1) * 512],
                    start=(k == 0),
                    stop=(k == KT - 1),
                )
        rms = nsb.tile([1, 2048], f32, tag="rms")
        nc.scalar.activation(
            out=rms[:], in_=ssq[:].rearrange("p a b -> p (a b)"),
            func=AF.Sqrt, bias=epsb[:1, :], scale=1.0 / D,
        )
        rrow = nsb.tile([1, 2048], f32, tag="rrow")
        nc.vector.reciprocal(out=rrow[:], in_=rms[:])
        bc = nps.tile([128, 4, 512], f32)
        for ch in range(4):
            nc.tensor.matmul(
                out=bc[:, ch, :],
                lhsT=ones_row[:],
                rhs=rrow[:, ch * 512 : (ch + 1) * 512],
                start=True, stop=True,
            )
        rmsb = nsb.tile([128, 2048], f16, tag="rmsb")
        nc.scalar.copy(out=rmsb[:], in_=bc[:].rearrange("p a b -> p (a b)"))
        nc.vector.tensor_tensor(
            out=hT[:], in0=xT[:],
            in1=rmsb[:].unsqueeze(1).broadcast_to([128, KT, 2048]),
            op=ALU.mult,
        )
        nc.vector.tensor_tensor(
            out=hT[:], in0=hT[:],
            in1=ncol[:].unsqueeze(2).broadcast_to([128, KT, 2048]),
            op=ALU.mult,
        )


def _allreduce_add(nc, tc, dram, name, psum_feed, xT):
    """psum_feed(ot) -> psum [128,2048] f32; AllReduce over cores; xT += result."""
    arin = dram.tile([KT, 128, 2048], f32, tag=f"{name}_in")
    arout = dram.tile([KT, 128, 2048], f32, tag=f"{name}_out", addr_space="Shared")
    with tc.tile_pool(name=f"{name}_st", bufs=2) as stp:
        for ot in range(KT):
            ps = psum_feed(ot)
            st = stp.tile([128, 2048], f32, tag="st")
            nc.scalar.copy(out=st[:], in_=ps)
            nc.sync.dma_start(out=arin[ot, :, :], in_=st[:])
        nc.gpsimd.collective_compute(
            "AllReduce", ALU.add,
            replica_groups=[list(range(N_CORES))],
            ins=[arin[:]], outs=[arout[:]],
        )
        for ot in range(KT):
            rd = stp.tile([128, 2048], f32, tag="st")
            nc.sync.dma_start(out=rd[:], in_=arout[ot, :, :])
            nc.vector.tensor_add(out=xT[:, ot, :], in0=xT[:, ot, :], in1=rd[:])


def _build_nc():
    nc = bass.Bass(num_devices=N_CORES)
    emb_c = nc.declare_dram_parameter("emb_c", [TOK, D // 4], u8, isOutput=False)
    emb_sc = nc.declare_dram_parameter("emb_sc", [TOK, KT], f32, isOutput=False)
    wqkv = nc.declare_dram_parameter("wqkv", [L, 3, 128, D // 4], u8, isOutput=False)
    wqkv_s = nc.declare_dram_parameter("wqkv_s", [L, 3, 128, KT], f32, isOutput=False)
    wo = nc.declare_dram_parameter("wo", [L, D, 32], u8, isOutput=False)
    wo_s = nc.declare_dram_parameter("wo_s", [L, D], f32, isOutput=False)
    wgu = nc.declare_dram_parameter("wgu", [L, 2, FSH, D // 4], u8, isOutput=False)
    wgu_s = nc.declare_dram_parameter("wgu_s", [L, 2, FSH, KT], f32, isOutput=False)
    wd = nc.declare_dram_parameter("wd", [L, D, FSH // 4], u8, isOutput=False)
    wd_s = nc.declare_dram_parameter("wd_s", [L, D, FSH // GS], f32, isOutput=False)
    na = nc.declare_dram_parameter("na", [L, D], f32, isOutput=False)
    nm = nc.declare_dram_parameter("nm", [L, D], f32, isOutput=False)
    fn = nc.declare_dram_parameter("fn", [D], f32, isOutput=False)
    alpha_p = nc.declare_dram_parameter("alpha_p", [L, 128], f32, isOutput=False)
    sel_p = nc.declare_dram_parameter("sel_p", [128, KT], f32, isOutput=False)
    mask_p = nc.declare_dram_parameter("mask_p", [128, 128], f32, isOutput=False)
    lm_t = nc.declare_dram_parameter("lm_t", [VSH, D // 4], u8, isOutput=False)
    lm_s = nc.declare_dram_parameter("lm_s", [VSH, KT], f32, isOutput=False)
    out = nc.declare_dram_parameter("logitsT", [VSH, TOK], f16, isOutput=True)

    with tile.TileContext(nc) as tc:
        with (
            tc.tile_pool(name="persist", bufs=1) as pp,
            tc.tile_pool(name="dram", bufs=2, space="DRAM") as dram,
        ):
            xT = pp.tile([128, KT, TOK], f32)
            hT = pp.tile([128, KT, TOK], f16)
            mask = pp.tile([128, 128], f32)
            nc.sync.dma_start(out=mask[:], in_=mask_p[:, :])
            sel = pp.tile([128, KT], f32)
            nc.sync.dma_start(out=sel[:], in_=sel_p[:, :])
            ones_col = pp.tile([128, 1], f16)
            nc.vector.memset(ones_col[:], 1.0)
            ones_row = pp.tile([1, 128], f32)
            nc.vector.memset(ones_row[:], 1.0)
            epsb = pp.tile([128, 1], f32)
            nc.vector.memset(epsb[:], 1e-6)
            shamt = pp.tile([128, 4], u8)
            for _pos in range(4):
                nc.vector.memset(shamt[:, _pos : _pos + 1], 2 * _pos)
            three = pp.tile([128, 1], u8)
            nc.vector.memset(three[:], 3)

            # embedding dequant on device (token-major) + transpose into hT,
            # then cast to f32 resident xT
            with tc.tile_pool(name="emb", bufs=3) as ebp:
                for tt in range(TOK // 128):
                    ec = ebp.tile([128, D // 4], u8, tag="ec")
                    nc.sync.dma_start(
                        out=ec[:], in_=emb_c[tt * 128 : (tt + 1) * 128, :]
                    )
                    ecu = ebp.tile([128, D], u8, tag="ecu")
                    for pos in range(4):
                        nc.vector.tensor_scalar(
                            out=ecu[:, pos * 256 : (pos + 1) * 256], in0=ec[:],
                            scalar1=shamt[:, pos : pos + 1], scalar2=three[:],
                            op0=ALU.logical_shift_right, op1=ALU.bitwise_and,
                        )
                    es = ebp.tile([128, KT], f32, tag="es")
                    nc.sync.dma_start(
                        out=es[:], in_=emb_sc[tt * 128 : (tt + 1) * 128, :]
                    )
                    edq = ebp.tile([128, D], f16, tag="edq")
                    esb = es[:].unsqueeze(2).broadcast_to([128, KT, 128])
                    nc.vector.tensor_tensor(
                        out=edq[:].rearrange("p (g k) -> p g k", g=KT),
                        in0=ecu[:].rearrange("p (g k) -> p g k", g=KT),
                        in1=esb, op=ALU.mult,
                    )
                    nc.vector.tensor_tensor(
                        out=edq[:].rearrange("p (g k) -> p g k", g=KT),
                        in0=edq[:].rearrange("p (g k) -> p g k", g=KT),
                        in1=esb, op=ALU.subtract,
                    )
                    nc.sync.dma_start_transpose(
                        out=hT[:, :, tt * 128 : (tt + 1) * 128], in_=edq[:]
                    )
            nc.vector.tensor_copy(out=xT[:], in_=hT[:])


            for li in range(L):
                # ---- attention block ----
                ncol = pp.tile([128, KT], f32, tag="ncol", bufs=2)
                nc.sync.dma_start(
                    out=ncol[:], in_=na[li, :].rearrange("(k p) -> p k", p=128)
                )
                _rmsnorm(nc, tc, epsb, xT, hT, ncol, ones_col, ones_row)

                with (
                    tc.tile_pool(name="att_sb", bufs=1) as asb,
                    tc.tile_pool(name="att_w", bufs=1) as awp,
                ):
                    # qkv weights -> WjT [128, KT, 128] per j
                    codes = awp.tile([128, 3, D // 4], u8, tag="c")
                    nc.sync.dma_start(
                        out=codes[:], in_=wqkv[li].rearrange("j p f -> p j f")
                    )
                    ucod = awp.tile([128, 3, D], u8, tag="uc")
                    for pos in range(4):
                        nc.vector.tensor_scalar(
                            out=ucod[:, :, pos * 256 : (pos + 1) * 256],
                            in0=codes[:],
                            scalar1=shamt[:, pos : pos + 1], scalar2=three[:],
                            op0=ALU.logical_shift_right, op1=ALU.bitwise_and,
                        )
                    scl = awp.tile([128, 3, KT], f32, tag="s")
                    nc.sync.dma_start(
                        out=scl[:], in_=wqkv_s[li].rearrange("j p f -> p j f")
                    )
                    wdq = awp.tile([128, 3, D], f16, tag="dq")
                    sclb = scl[:].unsqueeze(3).broadcast_to([128, 3, KT, 128])
                    nc.vector.tensor_tensor(
                        out=wdq[:].rearrange("p j (g k) -> p j g k", g=KT),
                        in0=ucod[:].rearrange("p j (g k) -> p j g k", g=KT),
                        in1=sclb, op=ALU.mult,
                    )
                    nc.vector.tensor_tensor(
                        out=wdq[:].rearrange("p j (g k) -> p j g k", g=KT),
                        in0=wdq[:].rearrange("p j (g k) -> p j g k", g=KT),
                        in1=sclb, op=ALU.subtract,
                    )
                    wT = asb.tile([128, 3, KT, 128], f16)
                    for j in range(3):
                        nc.sync.dma_start_transpose(
                            out=wT[:, j, :, :], in_=wdq[:, j, :]
                        )

                    qT = asb.tile([128, TOK], f16)
                    kTt = asb.tile([128, TOK], f16)
                    vT = asb.tile([128, TOK], f16)
                    with tc.tile_pool(name="qkv_ps", bufs=2, space="PSUM") as qps:
                        for j, dst in enumerate((qT, kTt, vT)):
                            ps = qps.tile([128, 4, 512], f32, tag="ps")
                            for ch in range(4):
                                for k in range(KT):
                                    nc.tensor.matmul(
                                        out=ps[:, ch, :],
                                        lhsT=wT[:, j, k, :],
                                        rhs=hT[:, k, ch * 512 : (ch + 1) * 512],
                                        start=(k == 0), stop=(k == KT - 1),
                                    )
                            if j == 0:
                                nc.scalar.mul(
                                    out=dst[:].rearrange("p (a b) -> p a b", a=4),
                                    in_=ps[:], mul=DH ** -0.5,
                                )
                            else:
                                nc.scalar.copy(
                                    out=dst[:].rearrange("p (a b) -> p a b", a=4),
                                    in_=ps[:],
                                )

                    # v natural layout per (b, h): v_nat[pair][k_loc, kt, dh]
                    v_nat = asb.tile([128, 2 * HL, KT, DH], f16)
                    for b in range(B):
                        for h in range(HL):
                            nc.sync.dma_start_transpose(
                                out=v_nat[:, b * HL + h, :, :],
                                in_=vT[h * DH : (h + 1) * DH,
                                       b * S : (b + 1) * S],
                            )

                    oT = asb.tile([128, TOK], f16)
                    with (
                        tc.tile_pool(name="sc_ps", bufs=1, space="PSUM") as scps,
                        tc.tile_pool(name="pv_ps", bufs=2, space="PSUM") as pvps,
                        tc.tile_pool(name="p_sb", bufs=2) as psb,
                    ):
                        for qi in range(8):
                            kext = (qi + 1) * 128
                            for b in range(B):
                                psc = scps.tile([128, 2, 1024], f32, tag="psc")
                                q0 = b * S + qi * 128
                                for h in range(HL):
                                    for c0 in range(0, kext, 512):
                                        cw = min(512, kext - c0)
                                        nc.tensor.matmul(
                                            out=psc[:, h, c0 : c0 + cw],
                                            lhsT=qT[h * DH : (h + 1) * DH,
                                                    q0 : q0 + 128],
                                            rhs=kTt[h * DH : (h + 1) * DH,
                                                    b * S + c0 : b * S + c0 + cw],
                                            start=True, stop=True,
                                        )
                                nc.vector.tensor_tensor(
                                    out=psc[:, :, qi * 128 : kext],
                                    in0=psc[:, :, qi * 128 : kext],
                                    in1=mask[:].unsqueeze(1).broadcast_to(
                                        [128, 2, 128]),
                                    op=ALU.add,
                                )
                                mx = psb.tile([128, 2], f32, tag="mx")
                                nc.vector.tensor_reduce(
                                    out=mx[:], in_=psc[:, :, :kext],
                                    axis=AX.X, op=ALU.max,
                                )
                                nc.vector.tensor_tensor(
                                    out=psc[:, :, :kext], in0=psc[:, :, :kext],
                                    in1=mx[:].unsqueeze(2).broadcast_to(
                                        [128, 2, kext]),
                                    op=ALU.subtract,
                                )
                                pex = psb.tile([128, 2, 1024], f16, tag="pex")
                                nc.scalar.activation(
                                    out=pex[:, :, :kext], in_=psc[:, :, :kext],
                                    func=AF.Exp,
                                )
                                sme = psb.tile([128, 2], f32, tag="sme")
                                nc.vector.tensor_reduce(
                                    out=sme[:], in_=pex[:, :, :kext],
                                    axis=AX.X, op=ALU.add,
                                )
                                rec = psb.tile([128, 2], f32, tag="rec")
                                nc.vector.reciprocal(out=rec[:], in_=sme[:])
                                nc.vector.tensor_tensor(
                                    out=pex[:, :, :kext], in0=pex[:, :, :kext],
                                    in1=rec[:].unsqueeze(2).broadcast_to(
                                        [128, 2, kext]),
                                    op=ALU.mult,
                                )
                                for h in range(HL):
                                    pT = psb.tile([128, 8, 128], f16, tag="pT")
                                    nc.sync.dma_start_transpose(
                                        out=pT[:, : qi + 1, :],
                                        in_=pex[:, h, :kext],
                                    )
                                    pv = pvps.tile([DH, 128], f32, tag="pv")
                                    for kk in range(qi + 1):
                                        nc.tensor.matmul(
                                            out=pv[:],
                                            lhsT=v_nat[:, b * HL + h, kk, :],
                                            rhs=pT[:, kk, :],
                                            start=(kk == 0), stop=(kk == qi),
                                        )
                                    nc.scalar.copy(
                                        out=oT[h * DH : (h + 1) * DH,
                                               q0 : q0 + 128],
                                        in_=pv[:],
                                    )

                    # alpha residual: oT += alpha_col * h_block(core)
                    acol = asb.tile([128, 1], f32)
                    nc.sync.dma_start(out=acol[:], in_=alpha_p[li, :].unsqueeze(1))
                    halp = asb.tile([128, TOK], f16)
                    nc.vector.tensor_scalar_mul(halp[:], hT[:, 0, :], sel[:, 0:1])
                    for k in range(1, KT):
                        nc.vector.scalar_tensor_tensor(
                            out=halp[:], in0=hT[:, k, :], scalar=sel[:, k : k + 1],
                            in1=halp[:], op0=ALU.mult, op1=ALU.add,
                        )
                    nc.vector.scalar_tensor_tensor(
                        out=oT[:], in0=halp[:], scalar=acol[:], in1=oT[:],
                        op0=ALU.mult, op1=ALU.add,
                    )

                    # O-projection partials -> AllReduce -> x update
                    ocodes = awp.tile([128, KT, 32], u8, tag="c")
                    nc.sync.dma_start(
                        out=ocodes[:],
                        in_=wo[li].rearrange("(ot p) i -> p ot i", p=128),
                    )
                    oucod = awp.tile([128, KT, 128], u8, tag="uc")
                    for pos in range(4):
                        nc.vector.tensor_scalar(
                            out=oucod[:, :, pos * 32 : (pos + 1) * 32],
                            in0=ocodes[:],
                            scalar1=shamt[:, pos : pos + 1], scalar2=three[:],
                            op0=ALU.logical_shift_right, op1=ALU.bitwise_and,
                        )
                    oscl = awp.tile([128, KT], f32, tag="s")
                    nc.sync.dma_start(
                        out=oscl[:], in_=wo_s[li].rearrange("(ot p) -> p ot", p=128)
                    )
                    odq = awp.tile([128, KT, 128], f16, tag="dq")
                    osclb = oscl[:].unsqueeze(2).broadcast_to([128, KT, 128])
                    nc.vector.tensor_tensor(
                        out=odq[:], in0=oucod[:], in1=osclb, op=ALU.mult,
                    )
                    nc.vector.tensor_tensor(
                        out=odq[:], in0=odq[:], in1=osclb, op=ALU.subtract,
                    )
                    woT = asb.tile([128, KT, 128], f16)
                    for ot in range(KT):
                        nc.sync.dma_start_transpose(
                            out=woT[:, ot, :], in_=odq[:, ot, :]
                        )
                    with tc.tile_pool(name="op_ps", bufs=2, space="PSUM") as ops:
                        def feed_o(ot):
                            ps = ops.tile([128, 4, 512], f32, tag="ps")
                            for ch in range(4):
                                nc.tensor.matmul(
                                    out=ps[:, ch, :],
                                    lhsT=woT[:, ot, :],
                                    rhs=oT[:, ch * 512 : (ch + 1) * 512],
                                    start=True, stop=True,
                                )
                            return ps[:].rearrange("p a b -> p (a b)")
                        _allreduce_add(nc, tc, dram, f"ar_o{li}", feed_o, xT)

                # ---- MLP block ----
                ncol2 = pp.tile([128, KT], f32, tag="ncol", bufs=2)
                nc.sync.dma_start(
                    out=ncol2[:], in_=nm[li, :].rearrange("(k p) -> p k", p=128)
                )
                _rmsnorm(nc, tc, epsb, xT, hT, ncol2, ones_col, ones_row)

                with (
                    tc.tile_pool(name="mlp_sb", bufs=1) as msb,
                    tc.tile_pool(name="mlp_w", bufs=1) as mwp,
                ):
                    guT = msb.tile([128, 2, KT, FSH], f16)
                    for j in range(2):
                        gcodes = mwp.tile([128, 4, D // 4], u8, tag="c")
                        nc.sync.dma_start(
                            out=gcodes[:],
                            in_=wgu[li, j].rearrange("(ot p) f -> p ot f", p=128),
                        )
                        gucod = mwp.tile([128, 4, D], u8, tag="uc")
                        for pos in range(4):
                            nc.vector.tensor_scalar(
                                out=gucod[:, :, pos * 256 : (pos + 1) * 256],
                                in0=gcodes[:],
                                scalar1=shamt[:, pos : pos + 1], scalar2=three[:],
                                op0=ALU.logical_shift_right, op1=ALU.bitwise_and,
                            )
                        gscl = mwp.tile([128, 4, KT], f32, tag="s")
                        nc.sync.dma_start(
                            out=gscl[:],
                            in_=wgu_s[li, j].rearrange("(ot p) f -> p ot f", p=128),
                        )
                        gdq = mwp.tile([128, 4, D], f16, tag="dq")
                        gsclb = gscl[:].unsqueeze(3).broadcast_to([128, 4, KT, 128])
                        nc.vector.tensor_tensor(
                            out=gdq[:].rearrange("p o (g k) -> p o g k", g=KT),
                            in0=gucod[:].rearrange("p o (g k) -> p o g k", g=KT),
                            in1=gsclb, op=ALU.mult,
                        )
                        nc.vector.tensor_tensor(
                            out=gdq[:].rearrange("p o (g k) -> p o g k", g=KT),
                            in0=gdq[:].rearrange("p o (g k) -> p o g k", g=KT),
                            in1=gsclb, op=ALU.subtract,
                        )
                        for ot in range(4):
                            nc.sync.dma_start_transpose(
                                out=guT[:, j, :, ot * 128 : (ot + 1) * 128],
                                in_=gdq[:, ot, :],
                            )
                    gT = msb.tile([128, 4, TOK], f16)
                    uT = msb.tile([128, 4, TOK], f16)
                    with tc.tile_pool(name="gu_ps", bufs=2, space="PSUM") as gps:
                        for j, dst in enumerate((gT, uT)):
                            for mb in range(4):
                                ps = gps.tile([128, 4, 512], f32, tag="ps")
                                for ch in range(4):
                                    for k in range(KT):
                                        nc.tensor.matmul(
                                            out=ps[:, ch, :],
                                            lhsT=guT[:, j, k,
                                                     mb * 128 : (mb + 1) * 128],
                                            rhs=hT[:, k, ch * 512 : (ch + 1) * 512],
                                            start=(k == 0), stop=(k == KT - 1),
                                        )
                                nc.scalar.copy(
                                    out=dst[:, mb, :].rearrange(
                                        "p (a b) -> p a b", a=4),
                                    in_=ps[:],
                                )
                    nc.scalar.activation(out=gT[:], in_=gT[:], func=AF.Silu)
                    nc.vector.tensor_mul(out=gT[:], in0=gT[:], in1=uT[:])

                    dcodes = mwp.tile([128, KT, FSH // 4], u8, tag="c")
                    nc.sync.dma_start(
                        out=dcodes[:],
                        in_=wd[li].rearrange("(ot p) m -> p ot m", p=128),
                    )
                    ducod = mwp.tile([128, KT, FSH], u8, tag="uc")
                    for pos in range(4):
                        nc.vector.tensor_scalar(
                            out=ducod[:, :, pos * 128 : (pos + 1) * 128],
                            in0=dcodes[:],
                            scalar1=shamt[:, pos : pos + 1], scalar2=three[:],
                            op0=ALU.logical_shift_right, op1=ALU.bitwise_and,
                        )
                    dscl = mwp.tile([128, KT, 4], f32, tag="s")
                    nc.sync.dma_start(
                        out=dscl[:],
                        in_=wd_s[li].rearrange("(ot p) g -> p ot g", p=128),
                    )
                    ddq = mwp.tile([128, KT, FSH], f16, tag="dq")
                    dsclb = dscl[:].unsqueeze(3).broadcast_to([128, KT, 4, 128])
                    nc.vector.tensor_tensor(
                        out=ddq[:].rearrange("p o (g k) -> p o g k", g=4),
                        in0=ducod[:].rearrange("p o (g k) -> p o g k", g=4),
                        in1=dsclb, op=ALU.mult,
                    )
                    nc.vector.tensor_tensor(
                        out=ddq[:].rearrange("p o (g k) -> p o g k", g=4),
                        in0=ddq[:].rearrange("p o (g k) -> p o g k", g=4),
                        in1=dsclb, op=ALU.subtract,
                    )
                    wdT = msb.tile([128, 4, KT, 128], f16)
                    for ot in range(KT):
                        nc.sync.dma_start_transpose(
                            out=wdT[:, :, ot, :], in_=ddq[:, ot, :]
                        )
                    with tc.tile_pool(name="dn_ps", bufs=2, space="PSUM") as dps:
                        def feed_d(ot):
                            ps = dps.tile([128, 4, 512], f32, tag="ps")
                            for ch in range(4):
                                for k in range(4):
                                    nc.tensor.matmul(
                                        out=ps[:, ch, :],
                                        lhsT=wdT[:, k, ot, :],
                                        rhs=gT[:, k, ch * 512 : (ch + 1) * 512],
                                        start=(k == 0), stop=(k == 3),
                                    )
                            return ps[:].rearrange("p a b -> p (a b)")
                        _allreduce_add(nc, tc, dram, f"ar_d{li}", feed_d, xT)

            # ---- final norm + LM head ----
            fcol = pp.tile([128, KT], f32, tag="ncol", bufs=2)
            nc.sync.dma_start(out=fcol[:], in_=fn[:].rearrange("(k p) -> p k", p=128))
            _rmsnorm(nc, tc, epsb, xT, hT, fcol, ones_col, ones_row)

            with (
                tc.tile_pool(name="lm_w", bufs=3) as lwp,
                tc.tile_pool(name="lm_o", bufs=3) as lop,
                tc.tile_pool(name="lm_ps", bufs=2, space="PSUM") as lps,
            ):
                nvc = (VSH + 127) // 128
                for vc in range(nvc):
                    r0 = vc * 128
                    nr = min(128, VSH - r0)
                    lcodes = lwp.tile([128, D // 4], u8, tag="lm_c")
                    nc.sync.dma_start(out=lcodes[:nr, :], in_=lm_t[r0 : r0 + nr, :])
                    lucod = lwp.tile([128, D], u8, tag="lm_uc")
                    for pos in range(4):
                        nc.vector.tensor_scalar(
                            out=lucod[:nr, pos * 256 : (pos + 1) * 256],
                            in0=lcodes[:nr, :],
                            scalar1=shamt[:nr, pos : pos + 1], scalar2=three[:nr],
                            op0=ALU.logical_shift_right, op1=ALU.bitwise_and,
                        )
                    lscl = lwp.tile([128, KT], f32, tag="lm_s")
                    nc.sync.dma_start(out=lscl[:nr, :], in_=lm_s[r0 : r0 + nr, :])
                    ldq = lwp.tile([128, D], f16, tag="lm_dq")
                    lsclb = lscl[:nr, :].unsqueeze(2).broadcast_to([nr, KT, 128])
                    nc.vector.tensor_tensor(
                        out=ldq[:nr, :].rearrange("p (g k) -> p g k", g=KT),
                        in0=lucod[:nr, :].rearrange("p (g k) -> p g k", g=KT),
                        in1=lsclb, op=ALU.mult,
                    )
                    nc.vector.tensor_tensor(
                        out=ldq[:nr, :].rearrange("p (g k) -> p g k", g=KT),
                        in0=ldq[:nr, :].rearrange("p (g k) -> p g k", g=KT),
                        in1=lsclb, op=ALU.subtract,
                    )
                    lmT = lwp.tile([128, KT, 128], f16, tag="lm_T")
                    nc.sync.dma_start_transpose(
                        out=lmT[:, :, :nr], in_=ldq[:nr, :]
                    )
                    ps = lps.tile([128, 4, 512], f32, tag="ps")
                    for ch in range(4):
                        for k in range(KT):
                            nc.tensor.matmul(
                                out=ps[:nr, ch, :],
                                lhsT=lmT[:, k, :nr],
                                rhs=hT[:, k, ch * 512 : (ch + 1) * 512],
                                start=(k == 0), stop=(k == KT - 1),
                            )
                    ost = lop.tile([128, TOK], f16, tag="ost")
                    nc.scalar.copy(
                        out=ost[:nr, :].rearrange("p (a b) -> p a b", a=4),
                        in_=ps[:nr, :, :],
                    )
                    nc.sync.dma_start(
                        out=out[r0 : r0 + nr, :], in_=ost[:nr, :]
                    )

    _split_excess_waits(nc)
    return nc


_NC_CACHE = None


def _get_nc():
    global _NC_CACHE
    if _NC_CACHE is None:
        _NC_CACHE = _build_nc()
    return _NC_CACHE


# ----------------------------------------------------------------- host part


def _pack2(c):
    """Pack ternary codes 4-per-byte along the last axis, quarter-interleaved:
    byte j holds codes at last-axis positions {j, q+j, 2q+j, 3q+j} (q = N/4),
    so the device unpack (shift 2*pos) writes contiguous quarters."""
    u = (np.asarray(c, np.int8) + 1).astype(np.uint8)
    q = u.shape[-1] // 4
    return np.ascontiguousarray(
        u[..., 0:q] | (u[..., q:2 * q] << 2) | (u[..., 2 * q:3 * q] << 4)
        | (u[..., 3 * q:] << 6)
    )


def _prep_inputs(inputs):
    """Build per-core in_maps (host work: slicing, casts, embedding gather)."""
    gi = lambda k: np.asarray(inputs[k])
    ids = gi("input_ids").reshape(-1)                        # [2048]
    emb_t = gi("emb_t").astype(np.int8, copy=False)
    emb_s = gi("emb_s").astype(np.float32, copy=False).reshape(V, KT)
    emb_c = _pack2(emb_t[ids])                               # [2048, 256] u8
    emb_sc = np.ascontiguousarray(emb_s[ids])                # [2048, 8] f32

    wq_t = gi("wq_t").astype(np.int8, copy=False)
    wk_t = gi("wk_t").astype(np.int8, copy=False)
    wv_t = gi("wv_t").astype(np.int8, copy=False)
    wo_t = gi("wo_t").astype(np.int8, copy=False)
    wg_t = gi("wg_t").astype(np.int8, copy=False)
    wu_t = gi("wu_t").astype(np.int8, copy=False)
    wd_t = gi("wd_t").astype(np.int8, copy=False)
    wq_s = gi("wq_s").astype(np.float32, copy=False).reshape(L, D, KT)
    wk_s = gi("wk_s").astype(np.float32, copy=False).reshape(L, D, KT)
    wv_s = gi("wv_s").astype(np.float32, copy=False).reshape(L, D, KT)
    wo_s = gi("wo_s").astype(np.float32, copy=False).reshape(L, D, KT)
    wg_s = gi("wg_s").astype(np.float32, copy=False).reshape(L, DFF, KT)
    wu_s = gi("wu_s").astype(np.float32, copy=False).reshape(L, DFF, KT)
    wd_s = gi("wd_s").astype(np.float32, copy=False).reshape(L, D, DFF // GS)
    na_w = gi("na_w").astype(np.float32, copy=False)
    nm_w = gi("nm_w").astype(np.float32, copy=False)
    fn_w = gi("fn_w").astype(np.float32, copy=False)
    alpha = gi("alpha").astype(np.float32, copy=False)       # [L, H]
    lm_t = gi("lm_t").astype(np.int8, copy=False)
    lm_s = gi("lm_s").astype(np.float32, copy=False).reshape(V, KT)

    mask = np.where(
        np.arange(128)[None, :] <= np.arange(128)[:, None], 0.0, -1e30
    ).astype(np.float32)

    in_maps = []
    for c in range(N_CORES):
        r1 = slice(c * 128, (c + 1) * 128)
        rf = slice(c * FSH, (c + 1) * FSH)
        rv = slice(c * VSH, (c + 1) * VSH)
        sel = np.zeros((128, KT), np.float32)
        sel[:, c] = 1.0
        acol = alpha[:, HL * c : HL * (c + 1)].repeat(DH, axis=1)  # [L, 128]
        in_maps.append({
            "emb_c": emb_c, "emb_sc": emb_sc,
            "wqkv": _pack2(np.stack([wq_t[:, r1], wk_t[:, r1], wv_t[:, r1]], axis=1)),
            "wqkv_s": np.stack([wq_s[:, r1], wk_s[:, r1], wv_s[:, r1]], axis=1),
            "wo": _pack2(wo_t[:, :, r1]),
            "wo_s": np.ascontiguousarray(wo_s[:, :, c]),
            "wgu": _pack2(np.stack([wg_t[:, rf], wu_t[:, rf]], axis=1)),
            "wgu_s": np.stack([wg_s[:, rf], wu_s[:, rf]], axis=1),
            "wd": _pack2(wd_t[:, :, rf]),
            "wd_s": np.ascontiguousarray(wd_s[:, :, 4 * c : 4 * (c + 1)]),
            "na": na_w, "nm": nm_w, "fn": fn_w,
            "alpha_p": np.ascontiguousarray(acol),
            "sel_p": sel, "mask_p": mask,
            "lm_t": _pack2(lm_t[rv]), "lm_s": lm_s[rv],
        })
    return in_maps


# ----------------------------------------------------------------- runner


def _run_spmd_fast(nc, in_maps):
    """run_bass_via_pjrt equivalent, but the donated zero output buffers are
    created on-device (saves a 128MB host->device transfer) and the global
    output array is returned without per-core splitting."""
    import jax
    import jax.numpy as jnp
    from jax.sharding import Mesh, NamedSharding, PartitionSpec
    from jax.experimental.shard_map import shard_map
    from concourse import bass2jax

    bass2jax.install_neuronx_cc_hook()
    assert nc.dbg_addr is None or not nc.dbg_callbacks
    partition_name = nc.partition_id_tensor.name if nc.partition_id_tensor else None
    in_names, out_names, out_avals = [], [], []
    for alloc in nc.m.functions[0].allocations:
        if not isinstance(alloc, mybir.MemoryLocationSet):
            continue
        name = alloc.memorylocations[0].name
        if alloc.kind == "ExternalInput":
            if name != partition_name:
                in_names.append(name)
        elif alloc.kind == "ExternalOutput":
            out_names.append(name)
            out_avals.append(
                jax.core.ShapedArray(
                    tuple(alloc.tensor_shape), mybir.dt.np(alloc.dtype)
                )
            )
    n_params = len(in_names)
    n_outs = len(out_avals)
    bind_names = list(in_names) + list(out_names)
    if partition_name is not None:
        bind_names.append(partition_name)

    def _body(*args):
        operands = list(args)
        if partition_name is not None:
            operands.append(bass2jax.partition_id_tensor())
        outs = bass2jax._bass_exec_p.bind(
            *operands,
            out_avals=tuple(out_avals),
            in_names=tuple(bind_names),
            out_names=tuple(out_names),
            lowering_input_output_aliases=(),
            sim_require_finite=True,
            sim_require_nnan=True,
            nc=nc,
        )
        return tuple(outs)

    n_cores = len(in_maps)
    devices = jax.devices()[:n_cores]
    mesh = Mesh(np.asarray(devices), ("core",))
    spec = PartitionSpec("core")
    in_specs = (spec,) * (n_params + n_outs)
    donate = tuple(range(n_params, n_params + n_outs))
    sharded = jax.jit(
        shard_map(
            _body, mesh=mesh, in_specs=in_specs, out_specs=(spec,) * n_outs,
            check_rep=False,
        ),
        donate_argnums=donate, keep_unused=True,
    )
    concat_in = [
        np.concatenate([np.asarray(m[name]) for m in in_maps], axis=0)
        for name in in_names
    ]
    sh = NamedSharding(mesh, spec)
    zero_fn = jax.jit(
        lambda: tuple(
            jnp.zeros((n_cores * a.shape[0], *a.shape[1:]), a.dtype)
            for a in out_avals
        ),
        out_shardings=(sh,) * n_outs,
    )
    dzeros = zero_fn()
    din = [jax.device_put(a, sh) for a in concat_in]
    out_arrs = sharded(*din, *dzeros)
    return {name: np.asarray(out_arrs[i]) for i, name in enumerate(out_names)}


# ----------------------------------------------------------------- entry


def kernel(_trace=False, **inputs):
    global LAST_EXEC_NS
    in_maps = _prep_inputs(inputs)
    nc = _get_nc()
    try:
        outs = _run_spmd_fast(nc, in_maps)
        lt = outs["logitsT"]  # [8*VSH, TOK] f16
    except Exception:
        res = run_bass_kernel_spmd(
            nc, in_maps, list(range(N_CORES)), trace=bool(_trace)
        )
        if getattr(res, "exec_time_ns", None):
            LAST_EXEC_NS = res.exec_time_ns
        lt = np.concatenate(
            [np.asarray(res.results[c]["logitsT"]) for c in range(N_CORES)],
            axis=0,
        )
    # transpose while still 2 bytes/elem, then widen contiguously
    logits = np.ascontiguousarray(lt.T).astype(np.float32)  # [TOK, V]
    return logits.reshape(B, S, V)
